# revision 37
# baseline (speedup 1.0000x reference)
"""Trainium2 Bass kernel: LayerNorm -> QKV -> linear (elu+1) attention -> proj.

Data-parallel over batch: 8 batch elements, one per NeuronCore. All matmuls
in bf16 (fp32 accumulation in PSUM); LayerNorm statistics in fp32; the
projection bias is applied in fp32.

Self-contained: hardcodes shapes from the problem spec.
"""

import numpy as np
import ml_dtypes

from concourse import bass, bacc, tile, mybir
from concourse.bass import ts, ds
from concourse.bass_utils import run_bass_kernel_spmd

F32 = mybir.dt.float32
BF16 = mybir.dt.bfloat16
AF = mybir.ActivationFunctionType
ALU = mybir.AluOpType

# Problem shapes
N = 4096          # tokens per batch element
D = 768           # model dim
H = 12            # heads
HD = 64           # head dim
E3 = 3 * D        # qkv width
P = 128
KT = D // P       # 6 d-tiles
NT = N // P       # 32 token tiles
CH = 8            # token chunks of 512
TPC = NT // CH    # 4 token tiles per chunk
CW = N // CH      # 512 chunk width
HW = 256          # half-chunk width
LN_EPS = 1e-5
EPS = 1e-6

N_CORES = 8
LDW_SKIP = True


def _build(dbg: bool = False):
    """Build the single-core program (SPMD: same NEFF on all 8 cores)."""
    nc = bacc.Bacc("TRN2", target_bir_lowering=False, debug=False,
                   num_devices=N_CORES)

    x_d = nc.dram_tensor("x", [N, D], BF16, kind="ExternalInput").ap()
    wqkvT_d = nc.dram_tensor("wqkvT", [D, E3], BF16, kind="ExternalInput").ap()
    wprojT_d = nc.dram_tensor("wprojT", [D, D], BF16, kind="ExternalInput").ap()
    bias128_d = nc.dram_tensor("bias128", [P, D], F32, kind="ExternalInput").ap()
    out_d = nc.dram_tensor("out", [N, D], F32, kind="ExternalOutput").ap()

    from contextlib import ExitStack
    with tile.TileContext(nc) as tc, ExitStack() as stk:
        _kernel(tc, stk, nc, x_d, wqkvT_d, wprojT_d, bias128_d, out_d, dbg)

    nc.compile()
    return nc


def _rstd_dve(nc, stat, mvs, w, tag):
    """rsqrt(var+eps) on [P, w] via bit-trick seed + 2 Newton steps (DVE).
    (ACT Sqrt thrashes activation tables against Exp/Relu; GPSIMD adds too
    much cross-engine handoff latency in the xhat path.)"""
    I32 = mybir.dt.int32
    veps = stat.tile([P, w], F32, tag=tag + "_v")
    for i, mv in enumerate(mvs):
        nc.vector.tensor_scalar_add(veps[:, i:i + 1], mv[:, 1:2], LN_EPS)
    t1 = stat.tile([P, w], I32, tag=tag + "_t")
    nc.vector.tensor_scalar(t1[:], veps[:].bitcast(I32), 1, None,
                            op0=ALU.arith_shift_right)
    rstd = stat.tile([P, w], F32, tag=tag + "_r")
    nc.vector.tensor_scalar(rstd[:].bitcast(I32), t1[:], -1, 0x5F3759DF,
                            op0=ALU.mult, op1=ALU.add)
    for _ in range(2):
        a = stat.tile([P, w], F32, tag=tag + "_a")
        nc.vector.tensor_tensor(a[:], rstd[:], rstd[:], ALU.mult)
        nc.vector.tensor_tensor(a[:], a[:], veps[:], ALU.mult)
        nc.vector.tensor_scalar(a[:], a[:], -0.5, 1.5, op0=ALU.mult,
                                op1=ALU.add)
        nc.vector.tensor_tensor(rstd[:], rstd[:], a[:], ALU.mult)
    return rstd


def _kernel(tc, stk, nc, x_d, wqkvT_d, wprojT_d, bias128_d, out_d, dbg=False):
    def dump(name, tl, shape, dtype):
        if not dbg:
            return
        d = nc.dram_tensor("dbg_" + name, shape, dtype, kind="ExternalOutput").ap()
        nc.sync.dma_start(d, tl)

    from contextlib import ExitStack
    # Pool close order at the phase boundary must be LIFO on the global
    # pool stack: allocate stkC (q-side), then stkA (LN/kv side), then
    # stkB (persistent kv bank); close B, A, C in that order.
    consts = stk.enter_context(tc.tile_pool(name="consts", bufs=1))
    xTp = stk.enter_context(tc.tile_pool(name="xT", bufs=2 * 2 * KT))
    ph2 = stk.enter_context(tc.tile_pool(name="ph2", bufs=2))
    stkC = stk.enter_context(ExitStack())   # qpsp + evac
    stkA = stk.enter_context(ExitStack())   # kvps + LN-side pools
    stkB = stk.enter_context(ExitStack())   # kv_ps persistent bank
    pools = {
        "qpsp": stkC.enter_context(
            tc.tile_pool(name="qpsp", bufs=1, space="PSUM")),
        "evac": stkC.enter_context(tc.tile_pool(name="evac", bufs=4)),
    }

    # Prefetch the first chunks' x tiles before the big weight DMAs so the
    # LN chain starts immediately (DMA queues drain in priority order).
    x_prefetch = {}
    xpool = stkA.enter_context(tc.tile_pool(name="x", bufs=10))
    for t in range(2 * TPC):
        xt = xpool.tile([P, D], BF16)
        nc.sync.dma_start(xt[:], x_d[ts(t, P), :])
        x_prefetch[t] = xt

    # --- resident weights ---
    wqkvT = consts.tile([P, KT, E3], BF16)
    wq_r = wqkvT_d.rearrange("(kt p) e -> p kt e", p=P)
    for kt in range(KT):
        nc.sync.dma_start(wqkvT[:, kt], wq_r[:, kt])
    wprojT = consts.tile([P, KT, D], BF16)
    wp_r = wprojT_d.rearrange("(kt p) e -> p kt e", p=P)
    for kt in range(KT):
        nc.sync.dma_start(wprojT[:, kt], wp_r[:, kt])

    # bias broadcast [128, D] comes pre-tiled from the host
    bias_sb = consts.tile([P, D], F32)
    nc.sync.dma_start(bias_sb[:], bias128_d)

    # zero-row for psum-bank init matmul; ones for ksl2 broadcast
    zrow = consts.tile([1, 512], BF16)
    nc.vector.memset(zrow[:], 0.0)
    ones_bf = consts.tile([1, P], BF16)
    nc.vector.memset(ones_bf[:], 1.0)
    ones64 = consts.tile([P, 64], BF16)
    nc.vector.memset(ones64[:], 1.0)

    stat = stkA.enter_context(tc.tile_pool(name="stat", bufs=12))
    xhatp = stkA.enter_context(tc.tile_pool(name="xhat", bufs=5))
    kvps = stkA.enter_context(tc.tile_pool(name="kvps", bufs=2, space="PSUM"))
    dramp = stkA.enter_context(tc.tile_pool(name="dram", bufs=5, space="DRAM"))
    ppersist = stkB.enter_context(
        tc.tile_pool(name="ppersist", bufs=1, space="PSUM"))

    # --- kv accumulator ---
    # pair p = h//2 -> cols [65p, 65p+65), head parity s=h%2 -> partitions
    # [64s, 64s+64). col 64 of each head block = k_sum.
    kv_ps = ppersist.tile([P, 6 * 65], F32)
    # Init the whole kv bank with one start=True matmul writing zeros: sets
    # every has_written bit so the 12 interleaved accumulation chains below
    # can all run with start=False. (start=True clears the *bank's* bits, so
    # per-chain start flags would clobber each other.)
    nc.tensor.matmul(kv_ps[:], ones_bf[:], zrow[:, 0:6 * 65], start=True,
                     stop=False, skip_group_check=True)

    qT_all = consts.tile([P, KT, N], BF16)

    def elu1_ps(out_ap, ps_ap, tagpfx):
        """elu(x)+1 = min(exp(x),1) + relu(x); exp/relu on ACT from PSUM,
        combine on DVE in bf16 (2x perf modes)."""
        w = ps_ap.shape[-1]
        evac = pools["evac"]
        et = evac.tile([P, w], BF16, tag=tagpfx + "_e")
        nc.scalar.activation(et[:], ps_ap, AF.Exp)
        rt = evac.tile([P, w], BF16, tag=tagpfx + "_r")
        nc.scalar.activation(rt[:], ps_ap, AF.Relu)
        em = evac.tile([P, w], BF16, tag=tagpfx + "_m")
        nc.vector.tensor_scalar_min(em[:], et[:], 1.0)
        nc.vector.tensor_tensor(out_ap, em[:], rt[:], ALU.add)

    # ====== phase-2 pieces (defined early; c0's z/attn interleaves with
    # chunk 7's q-part, later chunks' interleave with proj) ======
    attnTs = {}
    p2pools = {}

    def emit_zrep(c, kt):
        qT = qT_all[:, :, ts(c, CW)]
        zr_ps = p2pools["zrp"].tile([P, CW], F32)
        nc.tensor.matmul(zr_ps[:], ksl2[:, kt], qT[:, kt],
                         start=True, stop=True)
        # zri = 1/(z_pre + EPS) (scalar LUT reciprocal; z only scales)
        zri = ph2.tile([P, CW], BF16, tag="zri")
        nc.scalar.add_instruction(mybir.InstActivation(
            name=nc.get_next_instruction_name(),
            func=AF.Reciprocal,
            ins=[nc.scalar.lower_ap(zr_ps[:]),
                 mybir.ImmediateValue(dtype=F32, value=EPS),
                 mybir.ImmediateValue(dtype=F32, value=1.0),
                 mybir.ImmediateValue(dtype=F32, value=0.0)],
            outs=[nc.scalar.lower_ap(zri[:])]))
        # q is dead after the z-scale: overwrite qT_all in place (saves a
        # full [P, KT, CW] SBUF ring)
        nc.vector.tensor_mul(qT[:, kt], qT[:, kt], zri[:])

    def emit_attn(c, kt):
        at_ps = p2pools["atps"].tile([P, CW], F32)
        nc.tensor.matmul(at_ps[:], kvbd[:, kt],
                         qT_all[:, kt, ts(c, CW)],
                         start=True, stop=True)
        nc.scalar.activation(attnTs[c][:, kt], at_ps[:], AF.Copy)

    def emit_proj_tt(c, tt):
        attnT = attnTs[c]
        t = c * TPC + tt
        o5 = p2pools["ops5"].tile([P, 512], F32)
        o2 = p2pools["ops2"].tile([P, 256], F32)
        for kt in range(KT):
            for j, (o_ps, w_) in enumerate(((o5, 512), (o2, 256))):
                mm = nc.tensor.matmul(
                    o_ps[:, 0:w_],
                    attnT[:, kt, ts(tt, P)],
                    wprojT[:, kt, ds(j * 512, w_)],
                    start=(kt == 0), stop=(kt == KT - 1))
                if j > 0 and LDW_SKIP:
                    mm.ldweights = False  # same stationary as j-1
        osb = ph2.tile([P, D], F32, tag="osb")
        nc.vector.tensor_tensor(osb[:, 0:512], o5[:], bias_sb[:, 0:512],
                                ALU.add)
        nc.vector.tensor_tensor(osb[:, 512:D], o2[:], bias_sb[:, 512:D],
                                ALU.add)
        nc.sync.dma_start(out_d[ts(t, P), :], osb[:])

    def new_chunk_ph2(c):
        attnTs[c] = ph2.tile([P, KT, CW], BF16, tag="attnT",
                             name=f"attnT_{c}")

    # ============ PHASE 1: LN, transpose, k/v, q, kv accumulation =========
    # Half-chunk (256-token) LN->bounce->transpose granularity: the first
    # k/v matmuls only need the first half's transposes, cutting startup
    # latency and smoothing the ramp.
    def emit_kv_tt(c, xTh, tt):
        kv3 = kvps.tile([P, 3 * 512], F32, tag="ph1ps")  # qkv cols [768, 2304)
        for kt in range(KT):
            for j in range(3):
                mm = nc.tensor.matmul(
                    kv3[:, ts(j, 512)],
                    xTh[tt // 2][kt][:, ts(tt % 2, P)],
                    wqkvT[:, kt, ds(D + j * 512, 512)],
                    start=(kt == 0), stop=(kt == KT - 1))
                if j > 0 and LDW_SKIP:
                    mm.ldweights = False  # same stationary as j-1
        # k = elu1(cols 0:768)
        ktile = pools["evac"].tile([P, D], BF16, tag="ktile")
        elu1_ps(ktile[:], kv3[:, 0:D], "eluk")
        # v' = [v_h | 1] per head: [128, 12, 65]
        vtile = pools["evac"].tile([P, H, HD + 1], BF16, tag="vtile")
        nc.vector.memset(vtile[:, :, HD:HD + 1], 1.0)
        nc.scalar.activation(
            vtile[:, :, 0:HD],
            kv3[:, D:2 * D].rearrange("p (h e) -> p h e", h=H),
            AF.Copy)
        if c == 0 and tt == 0:
            dump("ktile0", ktile[:], [P, D], BF16)
            dump("vtile0", vtile[:], [P, H, HD + 1], BF16)
        return ktile, vtile

    def emit_accum(c, tt, ktile, vtile):
        t = c * TPC + tt
        for h in range(H):
            p_, s_ = h // 2, h % 2
            nc.tensor.matmul(
                kv_ps[ds(64 * s_, 64), ds(65 * p_, 65)],
                ktile[:, ds(HD * h, HD)],
                vtile[:, h],
                start=False, stop=(t == NT - 1),
                skip_group_check=True,
                tile_position=(0, 64 * s_))

    for c in range(CH):
        # --- LN + bounce + transpose, per half-chunk ---
        xTh = []
        for half in range(2):
            mvs = []
            xts = []
            for i in range(2):
                t = c * TPC + 2 * half + i
                if t in x_prefetch:
                    xt = x_prefetch.pop(t)
                else:
                    xt = xpool.tile([P, D], BF16)
                    nc.sync.dma_start(xt[:], x_d[ts(t, P), :])
                xts.append(xt)
                st6 = stat.tile([P, 2, 6], F32)
                nc.vector.bn_stats(st6[:, 0], xt[:, 0:D // 2])
                nc.vector.bn_stats(st6[:, 1], xt[:, D // 2:D])
                mv = stat.tile([P, 2], F32)
                nc.vector.bn_aggr(mv[:], st6[:])
                mvs.append(mv)
            rstd = _rstd_dve(nc, stat, mvs, 2, "rs")
            xhat = xhatp.tile([P, 2, D], BF16)
            xh_dram = dramp.tile([HW, D], BF16)
            for i in range(2):
                nc.vector.tensor_scalar(xhat[:, i], xts[i][:],
                                        mvs[i][:, 0:1], rstd[:, i:i + 1],
                                        op0=ALU.subtract, op1=ALU.mult)
                nc.sync.dma_start(xh_dram[ts(i, P), :], xhat[:, i])
            hT = [xTp.tile([P, HW], BF16, tag="xTkt",
                           name=f"xT_{c}_{half}_{kt}") for kt in range(KT)]
            for kt in range(KT):
                nc.sync.dma_start_transpose(out=hT[kt][:],
                                            in_=xh_dram[:, ts(kt, P)])
            xTh.append(hT)

        # --- k/v fills with kv-accums staggered one tile behind (keeps
        # accums' engine-queue waits short; PE stays on runnable work) ---
        kv_parts = {}
        kv_parts[0] = emit_kv_tt(c, xTh, 0)
        kv_parts[1] = emit_kv_tt(c, xTh, 1)
        emit_accum(c, 0, *kv_parts[0])
        kv_parts[2] = emit_kv_tt(c, xTh, 2)
        emit_accum(c, 1, *kv_parts[1])
        kv_parts[3] = emit_kv_tt(c, xTh, 3)
        emit_accum(c, 2, *kv_parts[2])

        last = (c == CH - 1)
        # --- q, directly transposed: qT[dq, t] (weights stationary) ---
        fillers = []
        for m in range(KT):
            q_ps = pools["qpsp"].tile([P, 512], F32, tag="qps1")
            for half in range(2):
                for kt in range(KT):
                    nc.tensor.matmul(q_ps[:, ts(half, HW)],
                                     wqkvT[:, kt, ts(m, P)],
                                     xTh[half][kt][:],
                                     start=(kt == 0), stop=(kt == KT - 1))
            elu1_ps(qT_all[:, m, ts(c, CW)], q_ps[:], "eluq")
            if m == 0:
                emit_accum(c, 3, *kv_parts[3])
                if last:
                    # ---- phase 1.5: kv -> sbuf, ksl2, kvbd (overlaps the
                    # remaining q matmuls of the final chunk) ----
                    kv_sb = consts.tile([P, 6 * 65], BF16)
                    nc.vector.tensor_copy(kv_sb[:], kv_ps[:])
                    ksum_f = consts.tile([P, KT], F32)
                    nc.vector.tensor_copy(
                        ksum_f[:],
                        kv_ps[:].rearrange("p (kt w) -> p kt w", w=65)[:, :, 64])
                    dump("kv", kv_sb[:], [P, 6 * 65], BF16)
                    stkB.close()
                    # ksl2[d, kt, m]: ksum[d] where head(d)==head(m) in the
                    # kt block (fuses z matmul with head-replication)
                    ksl2 = consts.tile([P, KT, P], BF16)
                    nc.vector.memset(ksl2[:], 0.0)
                    # kvbd: block-diagonal kv per head pair (full PE util)
                    kvbd = consts.tile([P, KT, P], BF16)
                    nc.vector.memset(kvbd[:], 0.0)
                    for kt in range(KT):
                        for s_ in range(2):
                            sl = ds(64 * s_, 64)
                            nc.vector.tensor_scalar_mul(
                                ksl2[sl, kt, ds(64 * s_, 64)],
                                ones64[sl, :],
                                ksum_f[sl, kt:kt + 1])
                            nc.vector.tensor_copy(
                                kvbd[sl, kt, ds(64 * s_, 64)],
                                kv_sb[sl, ds(65 * kt, 64)])
                    stkA.close()
                    stkC.close()
                    pools["qpsp"] = stk.enter_context(
                        tc.tile_pool(name="qps2", bufs=1, space="PSUM"))
                    pools["evac"] = stk.enter_context(
                        tc.tile_pool(name="evac2", bufs=4))
                    p2pools["zrp"] = stk.enter_context(
                        tc.tile_pool(name="zrp", bufs=2, space="PSUM"))
                    p2pools["atps"] = stk.enter_context(
                        tc.tile_pool(name="atps", bufs=1, space="PSUM"))
                    new_chunk_ph2(0)
                    fillers = ([lambda kt=kt: emit_zrep(0, kt)
                                for kt in range(KT)] +
                               [lambda kt=kt: emit_attn(0, kt)
                                for kt in range(KT)])
            if last and m >= 1 and fillers:
                # hide chunk 0's z/attn latency chain in the q stalls
                for _ in range(3 if m >= 2 else 0):
                    if fillers:
                        fillers.pop(0)()
        for f in fillers:
            f()

    # proj PSUM: the 256-wide tail tiles pack two-per-bank (a [128,256]
    # matmul output only needs to stay within one bank)
    p2pools["ops5"] = stk.enter_context(
        tc.tile_pool(name="ops5", bufs=2, space="PSUM"))
    p2pools["ops2"] = stk.enter_context(
        tc.tile_pool(name="ops2", bufs=2, space="PSUM"))

    # ============ PHASE 2: remaining chunks (z/attn of c hidden under
    # proj of c-1) ==========================================================
    for c in range(1, CH):
        new_chunk_ph2(c)
        emit_zrep(c, 0)
        emit_zrep(c, 1)
        for tt in range(TPC):
            emit_proj_tt(c - 1, tt)
            if tt + 2 < KT:
                emit_zrep(c, tt + 2)
            emit_attn(c, tt)
        for kt in range(TPC, KT):
            emit_attn(c, kt)
    for tt in range(TPC):
        emit_proj_tt(CH - 1, tt)


_CACHE = {}


def _get_nc(dbg: bool = False):
    key = ("nc", dbg)
    if key not in _CACHE:
        _CACHE[key] = _build(dbg)
    return _CACHE[key]


def kernel(x, ln_gamma, ln_beta, w_qkv, w_proj, b_proj, trace=False, dbg=False):
    x = np.asarray(x, dtype=np.float32)
    ln_gamma = np.asarray(ln_gamma, dtype=np.float32)
    ln_beta = np.asarray(ln_beta, dtype=np.float32)
    w_qkv = np.asarray(w_qkv, dtype=np.float32)
    w_proj = np.asarray(w_proj, dtype=np.float32)
    b_proj = np.asarray(b_proj, dtype=np.float32)
    bsz = x.shape[0]
    assert x.shape == (bsz, N, D) and bsz == N_CORES

    # Fold LN affine into the qkv projection (exact algebra):
    #   y = xhat*gamma + beta  =>  qkv = xhat @ (gamma*W)^T + W@beta
    wq_eff = (w_qkv * ln_gamma[None, :])          # [E3, D]
    cqkv = w_qkv @ ln_beta                        # [E3]
    if bool(np.any(cqkv)):
        raise NotImplementedError(
            "nonzero W@beta path not wired into the device kernel")

    wqkvT = np.ascontiguousarray(wq_eff.T).astype(ml_dtypes.bfloat16)
    wprojT = np.ascontiguousarray(w_proj.T).astype(ml_dtypes.bfloat16)
    bias128 = np.ascontiguousarray(
        np.broadcast_to(b_proj.astype(np.float32), (P, D)))

    # If the caller's process pinned jax to cpu (common for reference
    # generation), re-discover the neuron/axon backend before the PJRT run.
    import jax
    if len(jax.devices()) < N_CORES:
        try:
            jax.config.update("jax_platforms", None)
            jax.clear_backends()
        except Exception:
            pass

    nc = _get_nc(dbg)
    in_maps = []
    for i in range(N_CORES):
        m = {"x": np.ascontiguousarray(x[i]).astype(ml_dtypes.bfloat16),
             "wqkvT": wqkvT, "wprojT": wprojT, "bias128": bias128}
        in_maps.append(m)

    res = run_bass_kernel_spmd(nc, in_maps, core_ids=list(range(N_CORES)),
                               trace=trace)
    out = np.stack([res.results[i]["out"] for i in range(N_CORES)], axis=0)
    if dbg:
        return out, res
    if trace:
        return out, res
    return out


# revision 38
# speedup vs baseline: 1.0113x; 1.0113x over previous
"""Trainium2 Bass kernel: LayerNorm -> QKV -> linear (elu+1) attention -> proj.

Data-parallel over batch: 8 batch elements, one per NeuronCore. All matmuls
in bf16 (fp32 accumulation in PSUM); LayerNorm statistics in fp32; the
projection bias is applied in fp32.

Self-contained: hardcodes shapes from the problem spec.
"""

import numpy as np
import ml_dtypes

from concourse import bass, bacc, tile, mybir
from concourse.bass import ts, ds
from concourse.bass_utils import run_bass_kernel_spmd

F32 = mybir.dt.float32
BF16 = mybir.dt.bfloat16
AF = mybir.ActivationFunctionType
ALU = mybir.AluOpType

# Problem shapes
N = 4096          # tokens per batch element
D = 768           # model dim
H = 12            # heads
HD = 64           # head dim
E3 = 3 * D        # qkv width
P = 128
KT = D // P       # 6 d-tiles
NT = N // P       # 32 token tiles
CH = 8            # token chunks of 512
TPC = NT // CH    # 4 token tiles per chunk
CW = N // CH      # 512 chunk width
HW = 256          # half-chunk width
LN_EPS = 1e-5
EPS = 1e-6

N_CORES = 8
LDW_SKIP = True


def _build(dbg: bool = False):
    """Build the single-core program (SPMD: same NEFF on all 8 cores)."""
    nc = bacc.Bacc("TRN2", target_bir_lowering=False, debug=False,
                   num_devices=N_CORES)

    x_d = nc.dram_tensor("x", [N, D], BF16, kind="ExternalInput").ap()
    wqkvT_d = nc.dram_tensor("wqkvT", [D, E3], BF16, kind="ExternalInput").ap()
    wprojT_d = nc.dram_tensor("wprojT", [D, D], BF16, kind="ExternalInput").ap()
    bias128_d = nc.dram_tensor("bias128", [P, D], F32, kind="ExternalInput").ap()
    out_d = nc.dram_tensor("out", [N, D], F32, kind="ExternalOutput").ap()

    from contextlib import ExitStack
    with tile.TileContext(nc) as tc, ExitStack() as stk:
        _kernel(tc, stk, nc, x_d, wqkvT_d, wprojT_d, bias128_d, out_d, dbg)

    nc.compile()
    return nc


def _rstd_dve(nc, stat, mvs, w, tag):
    """rsqrt(var+eps) on [P, w] via bit-trick seed + 2 Newton steps (DVE).
    (ACT Sqrt thrashes activation tables against Exp/Relu; GPSIMD adds too
    much cross-engine handoff latency in the xhat path.)"""
    I32 = mybir.dt.int32
    veps = stat.tile([P, w], F32, tag=tag + "_v")
    for i, mv in enumerate(mvs):
        nc.vector.tensor_scalar_add(veps[:, i:i + 1], mv[:, 1:2], LN_EPS)
    t1 = stat.tile([P, w], I32, tag=tag + "_t")
    nc.vector.tensor_scalar(t1[:], veps[:].bitcast(I32), 1, None,
                            op0=ALU.arith_shift_right)
    rstd = stat.tile([P, w], F32, tag=tag + "_r")
    nc.vector.tensor_scalar(rstd[:].bitcast(I32), t1[:], -1, 0x5F3759DF,
                            op0=ALU.mult, op1=ALU.add)
    for _ in range(2):
        a = stat.tile([P, w], F32, tag=tag + "_a")
        nc.vector.tensor_tensor(a[:], rstd[:], rstd[:], ALU.mult)
        nc.vector.tensor_tensor(a[:], a[:], veps[:], ALU.mult)
        nc.vector.tensor_scalar(a[:], a[:], -0.5, 1.5, op0=ALU.mult,
                                op1=ALU.add)
        nc.vector.tensor_tensor(rstd[:], rstd[:], a[:], ALU.mult)
    return rstd


def _kernel(tc, stk, nc, x_d, wqkvT_d, wprojT_d, bias128_d, out_d, dbg=False):
    def dump(name, tl, shape, dtype):
        if not dbg:
            return
        d = nc.dram_tensor("dbg_" + name, shape, dtype, kind="ExternalOutput").ap()
        nc.sync.dma_start(d, tl)

    from contextlib import ExitStack
    # Pool close order at the phase boundary must be LIFO on the global
    # pool stack: allocate stkC (q-side), then stkA (LN/kv side), then
    # stkB (persistent kv bank); close B, A, C in that order.
    consts = stk.enter_context(tc.tile_pool(name="consts", bufs=1))
    xTp = stk.enter_context(tc.tile_pool(name="xT", bufs=2 * 2 * KT))
    ph2 = stk.enter_context(tc.tile_pool(name="ph2", bufs=2))
    stkC = stk.enter_context(ExitStack())   # qpsp + evac
    stkA = stk.enter_context(ExitStack())   # kvps + LN-side pools
    stkB = stk.enter_context(ExitStack())   # kv_ps persistent bank
    pools = {
        "qpsp": stkC.enter_context(
            tc.tile_pool(name="qpsp", bufs=1, space="PSUM")),
        "evac": stkC.enter_context(tc.tile_pool(name="evac", bufs=4)),
    }

    # Prefetch the first chunks' x tiles before the big weight DMAs so the
    # LN chain starts immediately (DMA queues drain in priority order).
    x_prefetch = {}
    xpool = stkA.enter_context(tc.tile_pool(name="x", bufs=10))
    for t in range(2 * TPC):
        xt = xpool.tile([P, D], BF16)
        nc.sync.dma_start(xt[:], x_d[ts(t, P), :])
        x_prefetch[t] = xt

    # --- resident weights ---
    wqkvT = consts.tile([P, KT, E3], BF16)
    wq_r = wqkvT_d.rearrange("(kt p) e -> p kt e", p=P)
    for kt in range(KT):
        nc.sync.dma_start(wqkvT[:, kt], wq_r[:, kt])
    wprojT = consts.tile([P, KT, D], BF16)
    wp_r = wprojT_d.rearrange("(kt p) e -> p kt e", p=P)
    for kt in range(KT):
        nc.sync.dma_start(wprojT[:, kt], wp_r[:, kt])

    # bias broadcast [128, D] comes pre-tiled from the host
    bias_sb = consts.tile([P, D], F32)
    nc.sync.dma_start(bias_sb[:], bias128_d)

    # zero-row for psum-bank init matmul; ones for ksl2 broadcast
    zrow = consts.tile([1, 512], BF16)
    nc.vector.memset(zrow[:], 0.0)
    ones_bf = consts.tile([1, P], BF16)
    nc.vector.memset(ones_bf[:], 1.0)
    ones64 = consts.tile([P, 64], BF16)
    nc.vector.memset(ones64[:], 1.0)

    stat = stkA.enter_context(tc.tile_pool(name="stat", bufs=12))
    xhatp = stkA.enter_context(tc.tile_pool(name="xhat", bufs=5))
    kvps = stkA.enter_context(tc.tile_pool(name="kvps", bufs=2, space="PSUM"))
    dramp = stkA.enter_context(tc.tile_pool(name="dram", bufs=5, space="DRAM"))
    ppersist = stkB.enter_context(
        tc.tile_pool(name="ppersist", bufs=1, space="PSUM"))

    # --- kv accumulator ---
    # pair p = h//2 -> cols [65p, 65p+65), head parity s=h%2 -> partitions
    # [64s, 64s+64). col 64 of each head block = k_sum.
    kv_ps = ppersist.tile([P, 6 * 65], F32)
    # Init the whole kv bank with one start=True matmul writing zeros: sets
    # every has_written bit so the 12 interleaved accumulation chains below
    # can all run with start=False. (start=True clears the *bank's* bits, so
    # per-chain start flags would clobber each other.)
    nc.tensor.matmul(kv_ps[:], ones_bf[:], zrow[:, 0:6 * 65], start=True,
                     stop=False, skip_group_check=True)

    qT_all = consts.tile([P, KT, N], BF16)

    def elu1_ps(out_ap, ps_ap, tagpfx):
        """elu(x)+1 = min(exp(x),1) + relu(x); exp/relu on ACT from PSUM,
        combine on DVE in bf16 (2x perf modes)."""
        w = ps_ap.shape[-1]
        evac = pools["evac"]
        et = evac.tile([P, w], BF16, tag=tagpfx + "_e")
        nc.scalar.activation(et[:], ps_ap, AF.Exp)
        rt = evac.tile([P, w], BF16, tag=tagpfx + "_r")
        nc.scalar.activation(rt[:], ps_ap, AF.Relu)
        em = evac.tile([P, w], BF16, tag=tagpfx + "_m")
        nc.vector.tensor_scalar_min(em[:], et[:], 1.0)
        nc.vector.tensor_tensor(out_ap, em[:], rt[:], ALU.add)

    # ====== phase-2 pieces (defined early; c0's z/attn interleaves with
    # chunk 7's q-part, later chunks' interleave with proj) ======
    attnTs = {}
    p2pools = {}

    def emit_zrep(c, kt):
        qT = qT_all[:, :, ts(c, CW)]
        zr_ps = p2pools["zrp"].tile([P, CW], F32)
        nc.tensor.matmul(zr_ps[:], ksl2[:, kt], qT[:, kt],
                         start=True, stop=True)
        # zri = 1/(z_pre + EPS) (scalar LUT reciprocal; z only scales)
        zri = ph2.tile([P, CW], BF16, tag="zri")
        nc.scalar.add_instruction(mybir.InstActivation(
            name=nc.get_next_instruction_name(),
            func=AF.Reciprocal,
            ins=[nc.scalar.lower_ap(zr_ps[:]),
                 mybir.ImmediateValue(dtype=F32, value=EPS),
                 mybir.ImmediateValue(dtype=F32, value=1.0),
                 mybir.ImmediateValue(dtype=F32, value=0.0)],
            outs=[nc.scalar.lower_ap(zri[:])]))
        # q is dead after the z-scale: overwrite qT_all in place (saves a
        # full [P, KT, CW] SBUF ring)
        nc.vector.tensor_mul(qT[:, kt], qT[:, kt], zri[:])

    def emit_attn(c, kt):
        at_ps = p2pools["atps"].tile([P, CW], F32)
        nc.tensor.matmul(at_ps[:], kvbd[:, kt],
                         qT_all[:, kt, ts(c, CW)],
                         start=True, stop=True)
        nc.scalar.activation(attnTs[c][:, kt], at_ps[:], AF.Copy)

    def emit_proj_tt(c, tt):
        attnT = attnTs[c]
        t = c * TPC + tt
        o5 = p2pools["ops5"].tile([P, 512], F32)
        o2 = p2pools["ops2"].tile([P, 256], F32)
        for kt in range(KT):
            for j, (o_ps, w_) in enumerate(((o5, 512), (o2, 256))):
                mm = nc.tensor.matmul(
                    o_ps[:, 0:w_],
                    attnT[:, kt, ts(tt, P)],
                    wprojT[:, kt, ds(j * 512, w_)],
                    start=(kt == 0), stop=(kt == KT - 1))
                if j > 0 and LDW_SKIP:
                    mm.ldweights = False  # same stationary as j-1
        osb = ph2.tile([P, D], F32, tag="osb")
        nc.vector.tensor_tensor(osb[:, 0:512], o5[:], bias_sb[:, 0:512],
                                ALU.add)
        nc.vector.tensor_tensor(osb[:, 512:D], o2[:], bias_sb[:, 512:D],
                                ALU.add)
        nc.sync.dma_start(out_d[ts(t, P), :], osb[:])

    def new_chunk_ph2(c):
        attnTs[c] = ph2.tile([P, KT, CW], BF16, tag="attnT",
                             name=f"attnT_{c}")

    # ============ PHASE 1: LN, transpose, k/v, q, kv accumulation =========
    # Half-chunk (256-token) LN->bounce->transpose granularity: the first
    # k/v matmuls only need the first half's transposes, cutting startup
    # latency and smoothing the ramp.
    def emit_kv_tt(c, xTh, tt):
        kv3 = kvps.tile([P, 3 * 512], F32, tag="ph1ps")  # qkv cols [768, 2304)
        for kt in range(KT):
            for j in range(3):
                mm = nc.tensor.matmul(
                    kv3[:, ts(j, 512)],
                    xTh[tt // 2][kt][:, ts(tt % 2, P)],
                    wqkvT[:, kt, ds(D + j * 512, 512)],
                    start=(kt == 0), stop=(kt == KT - 1))
                if j > 0 and LDW_SKIP:
                    mm.ldweights = False  # same stationary as j-1
        # k = elu1(cols 0:768)
        ktile = pools["evac"].tile([P, D], BF16, tag="ktile")
        elu1_ps(ktile[:], kv3[:, 0:D], "eluk")
        # v' = [v_h | 1] per head: [128, 12, 65]
        vtile = pools["evac"].tile([P, H, HD + 1], BF16, tag="vtile")
        nc.vector.memset(vtile[:, :, HD:HD + 1], 1.0)
        nc.scalar.activation(
            vtile[:, :, 0:HD],
            kv3[:, D:2 * D].rearrange("p (h e) -> p h e", h=H),
            AF.Copy)
        if c == 0 and tt == 0:
            dump("ktile0", ktile[:], [P, D], BF16)
            dump("vtile0", vtile[:], [P, H, HD + 1], BF16)
        return ktile, vtile

    def emit_accum(c, tt, ktile, vtile):
        t = c * TPC + tt
        for h in range(H):
            p_, s_ = h // 2, h % 2
            nc.tensor.matmul(
                kv_ps[ds(64 * s_, 64), ds(65 * p_, 65)],
                ktile[:, ds(HD * h, HD)],
                vtile[:, h],
                start=False, stop=(t == NT - 1),
                skip_group_check=True,
                tile_position=(0, 64 * s_))

    def emit_ln(c):
        # --- LN + bounce + transpose, per half-chunk ---
        xTh = []
        for half in range(2):
            mvs = []
            xts = []
            for i in range(2):
                t = c * TPC + 2 * half + i
                if t in x_prefetch:
                    xt = x_prefetch.pop(t)
                else:
                    xt = xpool.tile([P, D], BF16)
                    nc.sync.dma_start(xt[:], x_d[ts(t, P), :])
                xts.append(xt)
                st6 = stat.tile([P, 2, 6], F32)
                nc.vector.bn_stats(st6[:, 0], xt[:, 0:D // 2])
                nc.vector.bn_stats(st6[:, 1], xt[:, D // 2:D])
                mv = stat.tile([P, 2], F32)
                nc.vector.bn_aggr(mv[:], st6[:])
                mvs.append(mv)
            rstd = _rstd_dve(nc, stat, mvs, 2, "rs")
            xhat = xhatp.tile([P, 2, D], BF16)
            xh_dram = dramp.tile([HW, D], BF16)
            for i in range(2):
                nc.vector.tensor_scalar(xhat[:, i], xts[i][:],
                                        mvs[i][:, 0:1], rstd[:, i:i + 1],
                                        op0=ALU.subtract, op1=ALU.mult)
                nc.sync.dma_start(xh_dram[ts(i, P), :], xhat[:, i])
            hT = [xTp.tile([P, HW], BF16, tag="xTkt",
                           name=f"xT_{c}_{half}_{kt}") for kt in range(KT)]
            for kt in range(KT):
                nc.sync.dma_start_transpose(out=hT[kt][:],
                                            in_=xh_dram[:, ts(kt, P)])
            xTh.append(hT)
        return xTh

    # LN runs 2 chunks ahead of the matmul pipeline: its DVE ops must sit
    # ahead of chunk c's elu combines in the in-order DVE queue, and its
    # transposes ahead in the xbar queue, or the PE starves on the ramp.
    xThs = {0: emit_ln(0), 1: emit_ln(1)}
    for c in range(CH):
        if c + 2 < CH:
            xThs[c + 2] = emit_ln(c + 2)
        xTh = xThs.pop(c)

        # --- k/v fills with kv-accums staggered one tile behind (keeps
        # accums' engine-queue waits short; PE stays on runnable work) ---
        kv_parts = {}
        kv_parts[0] = emit_kv_tt(c, xTh, 0)
        kv_parts[1] = emit_kv_tt(c, xTh, 1)
        emit_accum(c, 0, *kv_parts[0])
        kv_parts[2] = emit_kv_tt(c, xTh, 2)
        emit_accum(c, 1, *kv_parts[1])
        kv_parts[3] = emit_kv_tt(c, xTh, 3)
        emit_accum(c, 2, *kv_parts[2])

        last = (c == CH - 1)
        # --- q, directly transposed: qT[dq, t] (weights stationary) ---
        fillers = []
        for m in range(KT):
            q_ps = pools["qpsp"].tile([P, 512], F32, tag="qps1")
            for half in range(2):
                for kt in range(KT):
                    nc.tensor.matmul(q_ps[:, ts(half, HW)],
                                     wqkvT[:, kt, ts(m, P)],
                                     xTh[half][kt][:],
                                     start=(kt == 0), stop=(kt == KT - 1))
            elu1_ps(qT_all[:, m, ts(c, CW)], q_ps[:], "eluq")
            if m == 0:
                emit_accum(c, 3, *kv_parts[3])
                if last:
                    # ---- phase 1.5: kv -> sbuf, ksl2, kvbd (overlaps the
                    # remaining q matmuls of the final chunk) ----
                    kv_sb = consts.tile([P, 6 * 65], BF16)
                    nc.vector.tensor_copy(kv_sb[:], kv_ps[:])
                    ksum_f = consts.tile([P, KT], F32)
                    nc.vector.tensor_copy(
                        ksum_f[:],
                        kv_ps[:].rearrange("p (kt w) -> p kt w", w=65)[:, :, 64])
                    dump("kv", kv_sb[:], [P, 6 * 65], BF16)
                    stkB.close()
                    # ksl2[d, kt, m]: ksum[d] where head(d)==head(m) in the
                    # kt block (fuses z matmul with head-replication)
                    ksl2 = consts.tile([P, KT, P], BF16)
                    nc.vector.memset(ksl2[:], 0.0)
                    # kvbd: block-diagonal kv per head pair (full PE util)
                    kvbd = consts.tile([P, KT, P], BF16)
                    nc.vector.memset(kvbd[:], 0.0)
                    for kt in range(KT):
                        for s_ in range(2):
                            sl = ds(64 * s_, 64)
                            nc.vector.tensor_scalar_mul(
                                ksl2[sl, kt, ds(64 * s_, 64)],
                                ones64[sl, :],
                                ksum_f[sl, kt:kt + 1])
                            nc.vector.tensor_copy(
                                kvbd[sl, kt, ds(64 * s_, 64)],
                                kv_sb[sl, ds(65 * kt, 64)])
                    stkA.close()
                    stkC.close()
                    pools["qpsp"] = stk.enter_context(
                        tc.tile_pool(name="qps2", bufs=1, space="PSUM"))
                    pools["evac"] = stk.enter_context(
                        tc.tile_pool(name="evac2", bufs=4))
                    p2pools["zrp"] = stk.enter_context(
                        tc.tile_pool(name="zrp", bufs=2, space="PSUM"))
                    p2pools["atps"] = stk.enter_context(
                        tc.tile_pool(name="atps", bufs=1, space="PSUM"))
                    new_chunk_ph2(0)
                    fillers = ([lambda kt=kt: emit_zrep(0, kt)
                                for kt in range(KT)] +
                               [lambda kt=kt: emit_attn(0, kt)
                                for kt in range(KT)])
            if last and m >= 1 and fillers:
                # hide chunk 0's z/attn latency chain in the q stalls
                for _ in range(3 if m >= 2 else 0):
                    if fillers:
                        fillers.pop(0)()
        for f in fillers:
            f()

    # proj PSUM: the 256-wide tail tiles pack two-per-bank (a [128,256]
    # matmul output only needs to stay within one bank)
    p2pools["ops5"] = stk.enter_context(
        tc.tile_pool(name="ops5", bufs=2, space="PSUM"))
    p2pools["ops2"] = stk.enter_context(
        tc.tile_pool(name="ops2", bufs=2, space="PSUM"))

    # ============ PHASE 2: remaining chunks (z/attn of c hidden under
    # proj of c-1) ==========================================================
    for c in range(1, CH):
        new_chunk_ph2(c)
        emit_zrep(c, 0)
        emit_zrep(c, 1)
        for tt in range(TPC):
            emit_proj_tt(c - 1, tt)
            if tt + 2 < KT:
                emit_zrep(c, tt + 2)
            emit_attn(c, tt)
        for kt in range(TPC, KT):
            emit_attn(c, kt)
    for tt in range(TPC):
        emit_proj_tt(CH - 1, tt)


_CACHE = {}


def _get_nc(dbg: bool = False):
    key = ("nc", dbg)
    if key not in _CACHE:
        _CACHE[key] = _build(dbg)
    return _CACHE[key]


def kernel(x, ln_gamma, ln_beta, w_qkv, w_proj, b_proj, trace=False, dbg=False):
    x = np.asarray(x, dtype=np.float32)
    ln_gamma = np.asarray(ln_gamma, dtype=np.float32)
    ln_beta = np.asarray(ln_beta, dtype=np.float32)
    w_qkv = np.asarray(w_qkv, dtype=np.float32)
    w_proj = np.asarray(w_proj, dtype=np.float32)
    b_proj = np.asarray(b_proj, dtype=np.float32)
    bsz = x.shape[0]
    assert x.shape == (bsz, N, D) and bsz == N_CORES

    # Fold LN affine into the qkv projection (exact algebra):
    #   y = xhat*gamma + beta  =>  qkv = xhat @ (gamma*W)^T + W@beta
    wq_eff = (w_qkv * ln_gamma[None, :])          # [E3, D]
    cqkv = w_qkv @ ln_beta                        # [E3]
    if bool(np.any(cqkv)):
        raise NotImplementedError(
            "nonzero W@beta path not wired into the device kernel")

    wqkvT = np.ascontiguousarray(wq_eff.T).astype(ml_dtypes.bfloat16)
    wprojT = np.ascontiguousarray(w_proj.T).astype(ml_dtypes.bfloat16)
    bias128 = np.ascontiguousarray(
        np.broadcast_to(b_proj.astype(np.float32), (P, D)))

    # If the caller's process pinned jax to cpu (common for reference
    # generation), re-discover the neuron/axon backend before the PJRT run.
    import jax
    if len(jax.devices()) < N_CORES:
        try:
            jax.config.update("jax_platforms", None)
            jax.clear_backends()
        except Exception:
            pass

    nc = _get_nc(dbg)
    in_maps = []
    for i in range(N_CORES):
        m = {"x": np.ascontiguousarray(x[i]).astype(ml_dtypes.bfloat16),
             "wqkvT": wqkvT, "wprojT": wprojT, "bias128": bias128}
        in_maps.append(m)

    res = run_bass_kernel_spmd(nc, in_maps, core_ids=list(range(N_CORES)),
                               trace=trace)
    out = np.stack([res.results[i]["out"] for i in range(N_CORES)], axis=0)
    if dbg:
        return out, res
    if trace:
        return out, res
    return out
